# revision 35
# baseline (speedup 1.0000x reference)
"""Trainium2 Bass kernel for nn_AttPool (4-layer GNN + additive-attention pooling).

Strategy (data-parallel over graphs, 32 graphs per NeuronCore), fp8 edition:
  * Host re-lays-out the edge list as per-graph dense INTEGER adjacency
    (A + I)^T in fp8 e4m3 (counts <= 5, exactly representable), with the
    per-dst 1/deg normalization carried separately as an f32 broadcast row.
  * The two dominant matmul groups run as fp8 DoubleRow (2 K-tiles per
    instruction at 0.5 cycles/row = 4x bf16 FLOP rate):
      - aggT  = sum_c h_c^T @ At_c       (adjacency spmm, [feat, dst])
      - u_r   = sum_l hT_l,r^T @ attW_l  (attention scores)
    Layer-0 uses a hi+lo fp8 split of node_feat (host-computed, exact to
    fp8^2 ~ bf16) because raw-Gaussian fp8 quantization busts accuracy;
    tanh-activation layers tolerate single fp8 (measured 0.83% rel err).
  * Per layer: aggT (DoubleRow, fp8) -> aggT_sb = aggT * recip (DVE/Pool
    tensor_mul, folds the deg normalization as a free-dim broadcast) ->
    linT = convW_l^T @ aggT (bf16) -> hT = tanh(linT) written fp8 straight
    into the per-graph hTcat tile (score-path operand) -> h = 4 PE
    transposes of hT chunks (fp8 identity matmuls) -> evac PSUM->SBUF.
    The old `lin` matmuls + second tanh are gone: h is a transpose of hT.
  * Scores: t = tanh(u) (bf16), then one fused DVE tensor_tensor_reduce
    (t * v -> sum) per chunk replaces the gpsimd multiply + DVE reduce.
    attn = exp(s) unnormalized with accumulated Z; normalization deferred
    to the output head (per-partition activation scale).
  * Pooling / pT extraction / output head keep the baseline quad-PSUM
    scheme; pooling rhs reads the fp8 h tiles with a bf16 attnCol lhsT.
  * Software pipelining: graphs processed in pairs; the score pipeline for
    pair i is emitted between the conv layers of pair i+1, pooling for
    pair i during pair i+2's aggT matmuls.
"""

import numpy as np
import ml_dtypes

B, N, F = 256, 512, 128
NL = 4
D = 512
OUT = 128
NCORES = 8
GPC = B // NCORES  # graphs per core

BF16 = ml_dtypes.bfloat16
FP8 = ml_dtypes.float8_e4m3

_NC_CACHE = {}


def _build_nc():
    if "nc" in _NC_CACHE:
        return _NC_CACHE["nc"]

    import concourse.bacc as bacc
    import concourse.tile as tile
    import concourse.mybir as mybir

    f32 = mybir.dt.float32
    bf16 = mybir.dt.bfloat16
    f8 = mybir.dt.float8e4
    DR = mybir.MatmulPerfMode.DoubleRow

    nc = bacc.Bacc(None, target_bir_lowering=False)

    at_d = nc.dram_tensor("at", [GPC, 128, NL, D], f8, kind="ExternalInput")
    h0_d = nc.dram_tensor("h0", [GPC, 128, 2, NL, F], f8, kind="ExternalInput")
    recip_d = nc.dram_tensor("recipb", [GPC, 128, D], f32, kind="ExternalInput")
    convw_d = nc.dram_tensor("convw", [128, NL, F], bf16, kind="ExternalInput")
    attw_d = nc.dram_tensor("attw", [128, NL, D], f8, kind="ExternalInput")
    vcol_d = nc.dram_tensor("vcol", [128, NL, 16], f8, kind="ExternalInput")
    outw_d = nc.dram_tensor("outw", [128, NL, OUT], bf16, kind="ExternalInput")
    ident_d = nc.dram_tensor("ident", [128, 128], f8, kind="ExternalInput")
    out_d = nc.dram_tensor("out", [GPC, OUT], f32, kind="ExternalOutput")

    with tile.TileContext(nc) as tc:
        with (
            tc.tile_pool(name="singles", bufs=1) as singles,
        ):
            convw_sb = singles.tile([128, NL, F], bf16)
            attw_sb = singles.tile([128, NL, D], f8)
            vcol_sb = singles.tile([128, NL, 16], f8)
            outw_sb = singles.tile([128, NL, OUT], bf16)
            ident_sb = singles.tile([128, 128], f8)
            ones128f = singles.tile([128, 1], f32)
            nc.vector.memset(ones128f[:], 1.0)
            one11 = singles.tile([1, 1], bf16)
            nc.vector.memset(one11[:], 1.0)
            ones128b = singles.tile([128, 1], bf16)
            nc.vector.memset(ones128b[:], 1.0)
            warmact = singles.tile([1, 2], f32)
            nc.scalar.activation(
                warmact[0:1, 0:1],
                ones128f[0:1, :],
                mybir.ActivationFunctionType.Tanh,
            )
            nc.scalar.activation(
                warmact[0:1, 1:2],
                ones128f[0:1, :],
                mybir.ActivationFunctionType.Exp,
            )
            zparts = singles.tile([128, GPC], f32)
            pT_sb = singles.tile([128, 4 * GPC], bf16)
            zrecip = singles.tile([GPC, 1], f32)

            from contextlib import ExitStack

            with ExitStack() as stk:
                p_at = stk.enter_context(tc.tile_pool(name="at", bufs=7))
                p_h0 = stk.enter_context(tc.tile_pool(name="h0", bufs=7))
                p_rc = stk.enter_context(tc.tile_pool(name="rc", bufs=7))
                p_hT = stk.enter_context(tc.tile_pool(name="hT", bufs=8))
                p_h = stk.enter_context(tc.tile_pool(name="h", bufs=13))
                p_aggsb = stk.enter_context(tc.tile_pool(name="aggsb", bufs=6))
                p_t = stk.enter_context(tc.tile_pool(name="t", bufs=8))
                p_scr = stk.enter_context(tc.tile_pool(name="scr", bufs=4))
                p_sc = stk.enter_context(tc.tile_pool(name="sc", bufs=16))
                p_pu4 = stk.enter_context(tc.tile_pool(name="pu4", bufs=3))
                p_pqscr = stk.enter_context(tc.tile_pool(name="pqscr", bufs=3))
                ps_a = stk.enter_context(
                    tc.tile_pool(name="ps_a", bufs=3, space="PSUM")
                )
                ps_h = stk.enter_context(
                    tc.tile_pool(name="ps_h", bufs=2, space="PSUM")
                )
                ps_u = stk.enter_context(tc.tile_pool(name="ps_u", bufs=2, space="PSUM"))
                ps_quad = stk.enter_context(
                    tc.tile_pool(name="ps_quad", bufs=1, space="PSUM")
                )
                # hcur[gg] = current h tile [128, 4, F] fp8 (None -> use h0 split)
                hcur = {}
                h0s = {}
                ats = {}
                rcs = {}
                hTs = {}
                cats = {}

                state = {"quad": None, "pcol": None, "pps": [], "qi": 0, "pc": 0}
                pending = []

                def new_quad():
                    return ps_quad.tile([128, D], mybir.dt.float32, name="pquad")

                def pool_begin(ea, eb):
                    (ga, cats_a, attn_a) = ea
                    (gb, cats_b, attn_b) = eb
                    qa, qb = ga % 4, gb % 4
                    if qa == 0:
                        state["quad"] = new_quad()
                        nc.vector.memset(state["quad"][:], 0.0)
                    state["pps"].append((qa, attn_a, cats_a, qb, attn_b, cats_b, state["quad"]))

                def pool_chunk(r):
                    for (qa, attn_a, cats_a, qb, attn_b, cats_b, pooledquad) in state["pps"]:
                        for q, attnCol, cat in (
                            (qa, attn_a, cats_a),
                            (qb, attn_b, cats_b),
                        ):
                            nc.tensor.matmul(
                                pooledquad[32 * q : 32 * q + 1, :],
                                attnCol[:, r : r + 1],
                                cat[:, r, :, :],
                                start=(r == 0),
                                stop=(r == 3),
                                tile_position=(0, 32 * q),
                            )

                def pool_finish():
                    pps = state["pps"]
                    state["pps"] = []
                    for (qa, attn_a, cats_a, qb, attn_b, cats_b, pooledquad) in pps:
                        pool_tail(qb, pooledquad)

                def pool_tail(qb, pooledquad):
                    if qb != 3:
                        return
                    qi = state["qi"]
                    state["qi"] += 1
                    pu4_sb = p_pu4.tile([128, D], bf16, name="pu4_sb")
                    # scalar, not vector: pu4 is slack-tolerant (pT is only
                    # needed at the head) while vector carries the
                    # chain-critical casts/evacs
                    nc.scalar.activation(
                        pu4_sb[:], pooledquad[:], mybir.ActivationFunctionType.Copy
                    )
                    if qi == GPC // 4 - 1:
                        # drain-critical last quad: extract pT columns with
                        # K=1 matmuls on the (idle) PE instead of the serial
                        # XBAR transpose + gather chain
                        pcol_ps = ps_a.tile(
                            [128, D], mybir.dt.float32, tag="a", name="pcol_ps"
                        )
                        for q in range(4):
                            for c in range(4):
                                nc.tensor.matmul(
                                    pcol_ps[:, 4 * c + q : 4 * c + q + 1],
                                    pu4_sb[32 * q : 32 * q + 1, c * 128 : (c + 1) * 128],
                                    ones128b[32 * q : 32 * q + 1, :],
                                    start=(c == 0),
                                    stop=(c == 3),
                                    tile_position=(32 * q, 0),
                                )
                        pcol_sb = p_pqscr.tile(
                            [128, 16], bf16, tag="pcol", name="pcol_sb"
                        )
                        nc.vector.tensor_copy(pcol_sb[:], pcol_ps[:, 0:16])
                        state["pcol"] = pcol_sb
                    else:
                        scr_t = p_pqscr.tile(
                            [128, 4, 128], bf16, tag="pq", name="scr_t"
                        )
                        nc.sync.dma_start_transpose(scr_t[:], pu4_sb[:])
                        for c in range(4):
                            nc.sync.dma_start(
                                pT_sb[:, c * GPC + 4 * qi : c * GPC + 4 * qi + 4],
                                scr_t[:, c, 0:128:32],
                            )

                def emit_pool_pair(ea, eb):
                    pool_begin(ea, eb)
                    for r in range(4):
                        pool_chunk(r)
                    pool_finish()

                def conv_aggT(gg, l):
                    aggT_ps = ps_a.tile([128, D], mybir.dt.float32, tag="a", name="aggT_ps")
                    at_t = ats[gg]
                    if l == 0:
                        # hi/lo split of node features: 4 DoubleRow matmuls
                        h0t = h0s[gg]
                        k = 0
                        for part in range(2):
                            for c in (0, 2):
                                nc.tensor.matmul(
                                    aggT_ps[:],
                                    h0t[:, part, c : c + 2, :],
                                    at_t[:, c : c + 2, :],
                                    start=(k == 0),
                                    stop=(k == 3),
                                    perf_mode=mybir.MatmulPerfMode.DoubleRow,
                                )
                                k += 1
                    else:
                        catg_t, lprev = hcur[gg]
                        for c in (0, 2):
                            nc.tensor.matmul(
                                aggT_ps[:],
                                catg_t[:, c : c + 2, lprev, :],
                                at_t[:, c : c + 2, :],
                                start=(c == 0),
                                stop=(c == 2),
                                perf_mode=mybir.MatmulPerfMode.DoubleRow,
                            )
                    return aggT_ps

                def conv_cast(gg, aggT_ps):
                    aggT_sb = p_aggsb.tile([128, D], bf16, name="aggT_sb")
                    nc.vector.tensor_mul(aggT_sb[:], aggT_ps[:], rcs[gg][:])
                    return aggT_sb

                def conv_linT(gg, l, aggT_sb):
                    linT_ps = ps_a.tile([128, D], mybir.dt.float32, tag="a", name="linT_ps")
                    nc.tensor.matmul(
                        linT_ps[:],
                        convw_sb[:, l, :],
                        aggT_sb[:],
                        start=True,
                        stop=True,
                    )
                    return linT_ps

                def conv_tanh(gg, l, linT_ps):
                    if l == 0:
                        hTs[gg] = p_hT.tile([128, NL, D], mybir.dt.float8e4, tag="hT", name="hTcat")
                    nc.scalar.activation(
                        hTs[gg][:, l, :], linT_ps[:], mybir.ActivationFunctionType.Tanh
                    )

                def conv_transp(gg, l):
                    # fp8 transpose writes PSUM with element step 2
                    h_ps = ps_h.tile([128, 4, 256], mybir.dt.float8e4, name="h_ps")
                    for r in range(4):
                        nc.tensor.transpose(
                            h_ps[:, r, 0:256:2],
                            hTs[gg][:, l, r * 128 : (r + 1) * 128],
                            ident_sb[:],
                        )
                    return h_ps

                def conv_evac(gg, l, h_ps):
                    if l == 0:
                        cats[gg] = p_h.tile(
                            [128, 4, NL, 128], mybir.dt.float8e4, tag="h", name="catg"
                        )
                    nc.vector.tensor_copy(
                        cats[gg][:, :, l, :], h_ps[:, :, 0:256:2]
                    )
                    hcur[gg] = (cats[gg], l)

                def conv_step(gg, l):
                    aggT_ps = conv_aggT(gg, l)
                    aggT_sb = conv_cast(gg, aggT_ps)
                    linT_ps = conv_linT(gg, l, aggT_sb)
                    conv_tanh(gg, l, linT_ps)
                    h_ps = conv_transp(gg, l)
                    conv_evac(gg, l, h_ps)

                def emit_score_stage(prevpair, a):
                    # stage a computes attd-chunk a of uT = attW^T @ cat^T for
                    # both graphs of the previous pair; a==3 finishes scores
                    ppair, tTs = prevpair
                    for gg in ppair:
                        uT_ps = ps_u.tile([128, D], mybir.dt.float32, tag="u", name="uT_ps")
                        for l in (0, 2):
                            nc.tensor.matmul(
                                uT_ps[:],
                                attw_sb[:, l : l + 2, a * 128 : (a + 1) * 128],
                                hTs[gg][:, l : l + 2, :],
                                start=(l == 0),
                                stop=(l == 2),
                                perf_mode=mybir.MatmulPerfMode.DoubleRow,
                            )
                        if a == 0:
                            tTs[gg] = p_t.tile([128, NL, D], mybir.dt.float8e4, tag="t", name="tTcat")
                        nc.scalar.activation(
                            tTs[gg][:, a, :], uT_ps[:], mybir.ActivationFunctionType.Tanh
                        )
                        if a == 3:
                            # s16 = (16 v)^T tT as an fp8 s-row, then transpose
                            # into column layout for exp + pooling
                            s_ps = ps_u.tile([2, D], mybir.dt.float32, tag="u", name="s_ps")
                            for aa in (0, 2):
                                nc.tensor.matmul(
                                    s_ps[:],
                                    vcol_sb[:, aa : aa + 2, 0:2],
                                    tTs[gg][:, aa : aa + 2, :],
                                    start=(aa == 0),
                                    stop=(aa == 2),
                                    perf_mode=mybir.MatmulPerfMode.DoubleRow,
                                )
                            s_sb = p_scr.tile([1, D], bf16, tag="scr", name="s_sb")
                            nc.vector.tensor_copy(s_sb[:], s_ps[0:1, :])
                            sCol_ps = ps_u.tile([128, 8], bf16, tag="u", name="sCol_ps")
                            for r in range(4):
                                nc.tensor.transpose(
                                    sCol_ps[:, 2 * r : 2 * r + 1],
                                    s_sb[0:1, r * 128 : (r + 1) * 128],
                                    one11[:],
                                )
                            attnCol = p_sc.tile([128, 4], mybir.dt.float8e4, tag="ac", name="attnCol")
                            nc.scalar.activation(
                                attnCol[:],
                                sCol_ps[:, 0:8:2],
                                mybir.ActivationFunctionType.Exp,
                                scale=1.0 / 16.0,
                                accum_out=zparts[:, gg : gg + 1],
                            )
                            pending.append((gg, cats[gg], attnCol))

                prev = None
                # first two groups are quads: the fill has no score/pool
                # filler work, so deeper graph-overlap is free there
                groups = [tuple(range(0, 4)), tuple(range(4, 8))] + [
                    tuple(range(g, g + 3)) for g in range(8, GPC, 3)
                ]
                for gi, pair in enumerate(groups):
                    for gg in pair:
                        at_t = p_at.tile([128, NL, D], mybir.dt.float8e4, tag="at")
                        ats[gg] = at_t
                        h0t = p_h0.tile([128, 2, NL, F], mybir.dt.float8e4, tag="h0")
                        h0s[gg] = h0t
                        rc_t = p_rc.tile([128, D], mybir.dt.float32, tag="rc")
                        rcs[gg] = rc_t
                        if gg <= 1:
                            # first pair: split the big at DMA so pieces land
                            # on parallel queues
                            nc.sync.dma_start(at_t[:, 0:2, :], at_d[gg, :, 0:2, :])
                            nc.sync.dma_start(at_t[:, 2:4, :], at_d[gg, :, 2:4, :])
                        else:
                            nc.sync.dma_start(at_t[:], at_d[gg])
                        nc.sync.dma_start(h0t[:], h0_d[gg])
                        nc.sync.dma_start(rc_t[:], recip_d[gg])
                    if gi == 0:
                        nc.sync.dma_start(convw_sb[:], convw_d[:])
                        nc.sync.dma_start(ident_sb[:], ident_d[:])
                        nc.sync.dma_start(attw_sb[:], attw_d[:])
                        nc.sync.dma_start(vcol_sb[:], vcol_d[:])
                        nc.sync.dma_start(outw_sb[:], outw_d[:])

                    # ---------------- conv layers ----------------
                    # (gp==0 runs the same phased loop; prev=None skips the
                    # score stages and pending is empty so no pool work)
                    npool = 1 + (gi % 2)
                    for _ in range(npool):
                        if len(pending) > 3:
                            pool_begin(pending.pop(0), pending.pop(0))
                    for l in range(NL):
                        # phase-interleave the pair so PE never waits on one
                        # graph's cast/tanh chain; score + pool work for older
                        # graphs fills the remaining stalls.  One pool chunk
                        # (2 short matmuls) drips into each layer's evac-wait
                        # gap instead of a single 3us lump on the chain.
                        aps = {gg: conv_aggT(gg, l) for gg in pair}
                        sbs = {gg: conv_cast(gg, aps[gg]) for gg in pair}
                        lps = {gg: conv_linT(gg, l, sbs[gg]) for gg in pair}
                        for gg in pair:
                            conv_tanh(gg, l, lps[gg])
                        if prev is not None:
                            emit_score_stage(prev, l)
                        hps = {gg: conv_transp(gg, l) for gg in pair}
                        for gg in pair:
                            conv_evac(gg, l, hps[gg])
                        if state["pps"]:
                            pool_chunk(l)
                            if l == 3:
                                pool_finish()

                    prev = (pair, {})

                # drain: score the last pair, dripping pool chunks between
                def drain_pool_tick():
                    if not state["pps"] and len(pending) >= 2:
                        pool_begin(pending.pop(0), pending.pop(0))
                        state["pc"] = 0
                    if state["pps"]:
                        pool_chunk(state["pc"])
                        state["pc"] += 1
                        if state["pc"] == 4:
                            pool_finish()

                for r in range(4):
                    emit_score_stage(prev, r)
                    drain_pool_tick()
                    drain_pool_tick()
                while pending or state["pps"]:
                    drain_pool_tick()

                # ---------------- output head ----------------
                pq_fin = new_quad()
                zall_ps = pq_fin[0:GPC, 0:1]
                nc.tensor.matmul(
                    zall_ps, zparts[:], ones128f[:], start=True, stop=True
                )
                nc.vector.reciprocal(zrecip[:], zall_ps)
                # rows 28:31 read the last quad's pcol_sb directly (no pT
                # gather DMAs on the drain path); separate base-0 Z recip
                zallB_ps = pq_fin[0:4, 1:2]
                nc.tensor.matmul(
                    zallB_ps,
                    zparts[:, GPC - 4 : GPC],
                    ones128f[:],
                    start=True,
                    stop=True,
                )
                zrecipB = singles.tile([4, 1], f32)
                nc.vector.reciprocal(zrecipB[:], zallB_ps)
                out_psA = pq_fin[0:28, 2 * OUT : 3 * OUT]
                out_psB = pq_fin[0:4, 3 * OUT : 4 * OUT]
                pcol_fin = state["pcol"]
                for c in range(4):
                    nc.tensor.matmul(
                        out_psA,
                        pT_sb[:, c * GPC : c * GPC + 28],
                        outw_sb[:, c, :],
                        start=(c == 0),
                        stop=(c == 3),
                    )
                for c in range(4):
                    nc.tensor.matmul(
                        out_psB,
                        pcol_fin[:, 4 * c : 4 * c + 4],
                        outw_sb[:, c, :],
                        start=(c == 0),
                        stop=(c == 3),
                    )
                out_finA = singles.tile([GPC, OUT], f32)
                out_finB = singles.tile([4, OUT], f32)
                nc.scalar.activation(
                    out_finA[0:28, :],
                    out_psA,
                    mybir.ActivationFunctionType.Relu,
                    scale=zrecip[0:28, :],
                )
                nc.scalar.activation(
                    out_finB[:],
                    out_psB,
                    mybir.ActivationFunctionType.Relu,
                    scale=zrecipB[:],
                )
                nc.sync.dma_start(out_d[0:28], out_finA[0:28, :])
                nc.sync.dma_start(out_d[28:GPC], out_finB[:])

    nc.compile()
    _NC_CACHE["nc"] = nc
    return nc


def _prep_inputs(node_feat, edge_src, edge_dst, conv_W, att_W, att_v, out_W):
    src = edge_src.astype(np.int64)
    dst = edge_dst.astype(np.int64)
    ls = src - (dst // N) * N  # src local id within dst's graph
    idx = dst * N + ls
    counts = np.bincount(idx, minlength=B * N * N).astype(np.float32)
    A = counts.reshape(B, N, N)
    iN = np.arange(N)
    A[:, iN, iN] += 1.0
    if A.max() > 16:
        raise ValueError("adjacency counts exceed exact fp8 integer range")
    degs = A.sum(axis=2)  # == deg + 1
    At = np.ascontiguousarray(A.transpose(0, 2, 1))  # [g, src, dst]
    at_host = np.ascontiguousarray(
        At.reshape(B, 4, 128, N).transpose(0, 2, 1, 3)
    ).astype(FP8)  # [B, 128, 4, 512] integer counts, exact

    h0 = np.ascontiguousarray(
        node_feat.reshape(B, 4, 128, F).transpose(0, 2, 1, 3)
    )  # [B, 128, 4, 128]
    h0_hi = h0.astype(FP8)
    h0_lo = (h0 - h0_hi.astype(np.float32)).astype(FP8)
    h0_host = np.ascontiguousarray(
        np.stack([h0_hi, h0_lo], axis=2)
    )  # [B, 128, 2, 4, 128]

    recipb = np.ascontiguousarray(
        np.broadcast_to((1.0 / degs)[:, None, :], (B, 128, N))
    ).astype(np.float32)

    convw2 = np.ascontiguousarray(conv_W.transpose(1, 0, 2)).astype(BF16)
    attw2 = (
        np.ascontiguousarray(att_W.reshape(4, 128, D).transpose(1, 0, 2))
    ).astype(FP8)
    vcol = np.zeros((128, NL, 16), dtype=np.float32)
    vcol[:, :, 0] = (att_v * 16.0).reshape(4, 128).T
    vcol = vcol.astype(FP8)
    outw2 = (
        np.ascontiguousarray(out_W.reshape(4, 128, OUT).transpose(1, 0, 2))
    ).astype(BF16)
    ident = np.eye(128, dtype=np.float32).astype(FP8)
    return at_host, h0_host, recipb, convw2, attw2, vcol, outw2, ident


def _host_fallback(node_feat, edge_src, edge_dst, conv_W, conv_b, att_W, att_b,
                   att_v, out_W, out_b):
    # exact numpy mirror of the reference; only used if biases are nonzero
    src = edge_src.astype(np.int64)
    dst = edge_dst.astype(np.int64)
    ls = src - (dst // N) * N
    idx = dst * N + ls
    counts = np.bincount(idx, minlength=B * N * N).astype(np.float32)
    A = counts.reshape(B, N, N)
    iN = np.arange(N)
    A[:, iN, iN] += 1.0
    degs = A.sum(axis=2)[:, :, None]
    h = node_feat.reshape(B, N, F).astype(np.float32)
    cats = []
    for l in range(NL):
        agg = np.matmul(A, h)
        lin = np.matmul(agg, conv_W[l]) + conv_b[l]
        h = np.tanh(lin / degs)
        cats.append(h)
    cat = np.concatenate(cats, axis=2)
    u = np.matmul(cat, att_W) + att_b
    s = np.tanh(u) @ att_v
    s = s - s.max(axis=1, keepdims=True)
    a = np.exp(s)
    a /= a.sum(axis=1, keepdims=True)
    pooled = np.einsum('bn,bnd->bd', a, cat)
    return np.maximum(pooled @ out_W + out_b, 0.0).astype(np.float32)


def kernel(
    node_feat,
    edge_src,
    edge_dst,
    conv_W,
    conv_b,
    att_W,
    att_b,
    att_v,
    out_W,
    out_b,
):
    node_feat = np.asarray(node_feat, dtype=np.float32)
    edge_src = np.asarray(edge_src)
    edge_dst = np.asarray(edge_dst)
    conv_W = np.asarray(conv_W, dtype=np.float32)
    conv_b = np.asarray(conv_b, dtype=np.float32)
    att_W = np.asarray(att_W, dtype=np.float32)
    att_b = np.asarray(att_b, dtype=np.float32)
    att_v = np.asarray(att_v, dtype=np.float32)
    out_W = np.asarray(out_W, dtype=np.float32)
    out_b = np.asarray(out_b, dtype=np.float32)

    if np.any(conv_b) or np.any(att_b) or np.any(out_b):
        return _host_fallback(node_feat, edge_src, edge_dst, conv_W, conv_b,
                              att_W, att_b, att_v, out_W, out_b)

    from concourse.bass_utils import run_bass_kernel_spmd

    at_host, h0_host, recipb, convw2, attw2, vcol, outw2, ident = _prep_inputs(
        node_feat, edge_src, edge_dst, conv_W, att_W, att_v, out_W
    )

    nc = _build_nc()

    in_maps = []
    for c in range(NCORES):
        sl = slice(c * GPC, (c + 1) * GPC)
        in_maps.append({
            "at": at_host[sl],
            "h0": h0_host[sl],
            "recipb": recipb[sl],
            "convw": convw2,
            "attw": attw2,
            "vcol": vcol,
            "outw": outw2,
            "ident": ident,
        })

    res = run_bass_kernel_spmd(nc, in_maps, core_ids=list(range(NCORES)))
    out = np.concatenate([r["out"] for r in res.results], axis=0)
    return np.ascontiguousarray(out.astype(np.float32))


# revision 36
# speedup vs baseline: 1.0086x; 1.0086x over previous
"""Trainium2 Bass kernel for nn_AttPool (4-layer GNN + additive-attention pooling).

Strategy (data-parallel over graphs, 32 graphs per NeuronCore), fp8 edition:
  * Host re-lays-out the edge list as per-graph dense INTEGER adjacency
    (A + I)^T in fp8 e4m3 (counts <= 5, exactly representable), with the
    per-dst 1/deg normalization carried separately as an f32 broadcast row.
  * The two dominant matmul groups run as fp8 DoubleRow (2 K-tiles per
    instruction at 0.5 cycles/row = 4x bf16 FLOP rate):
      - aggT  = sum_c h_c^T @ At_c       (adjacency spmm, [feat, dst])
      - u_r   = sum_l hT_l,r^T @ attW_l  (attention scores)
    Layer-0 uses a hi+lo fp8 split of node_feat (host-computed, exact to
    fp8^2 ~ bf16) because raw-Gaussian fp8 quantization busts accuracy;
    tanh-activation layers tolerate single fp8 (measured 0.83% rel err).
  * Per layer: aggT (DoubleRow, fp8) -> aggT_sb = aggT * recip (DVE/Pool
    tensor_mul, folds the deg normalization as a free-dim broadcast) ->
    linT = convW_l^T @ aggT (bf16) -> hT = tanh(linT) written fp8 straight
    into the per-graph hTcat tile (score-path operand) -> h = 4 PE
    transposes of hT chunks (fp8 identity matmuls) -> evac PSUM->SBUF.
    The old `lin` matmuls + second tanh are gone: h is a transpose of hT.
  * Scores: t = tanh(u) (bf16), then one fused DVE tensor_tensor_reduce
    (t * v -> sum) per chunk replaces the gpsimd multiply + DVE reduce.
    attn = exp(s) unnormalized with accumulated Z; normalization deferred
    to the output head (per-partition activation scale).
  * Pooling / pT extraction / output head keep the baseline quad-PSUM
    scheme; pooling rhs reads the fp8 h tiles with a bf16 attnCol lhsT.
  * Software pipelining: graphs processed in pairs; the score pipeline for
    pair i is emitted between the conv layers of pair i+1, pooling for
    pair i during pair i+2's aggT matmuls.
"""

import numpy as np
import ml_dtypes

B, N, F = 256, 512, 128
NL = 4
D = 512
OUT = 128
NCORES = 8
GPC = B // NCORES  # graphs per core

BF16 = ml_dtypes.bfloat16
FP8 = ml_dtypes.float8_e4m3

_NC_CACHE = {}


def _build_nc():
    if "nc" in _NC_CACHE:
        return _NC_CACHE["nc"]

    import concourse.bacc as bacc
    import concourse.tile as tile
    import concourse.mybir as mybir

    f32 = mybir.dt.float32
    bf16 = mybir.dt.bfloat16
    f8 = mybir.dt.float8e4
    DR = mybir.MatmulPerfMode.DoubleRow

    nc = bacc.Bacc(None, target_bir_lowering=False)

    at_d = nc.dram_tensor("at", [GPC, 128, NL, D], f8, kind="ExternalInput")
    h0_d = nc.dram_tensor("h0", [GPC, 128, 2, NL, F], f8, kind="ExternalInput")
    recip_d = nc.dram_tensor("recipb", [GPC, 128, D], f32, kind="ExternalInput")
    convw_d = nc.dram_tensor("convw", [128, NL, F], bf16, kind="ExternalInput")
    attw_d = nc.dram_tensor("attw", [128, NL, D], f8, kind="ExternalInput")
    vcol_d = nc.dram_tensor("vcol", [128, NL, 16], f8, kind="ExternalInput")
    outw_d = nc.dram_tensor("outw", [128, NL, OUT], bf16, kind="ExternalInput")
    ident_d = nc.dram_tensor("ident", [128, 128], f8, kind="ExternalInput")
    out_d = nc.dram_tensor("out", [GPC, OUT], f32, kind="ExternalOutput")

    with tile.TileContext(nc) as tc:
        with (
            tc.tile_pool(name="singles", bufs=1) as singles,
        ):
            convw_sb = singles.tile([128, NL, F], bf16)
            attw_sb = singles.tile([128, NL, D], f8)
            vcol_sb = singles.tile([128, NL, 16], f8)
            outw_sb = singles.tile([128, NL, OUT], bf16)
            ident_sb = singles.tile([128, 128], f8)
            ones128f = singles.tile([128, 1], f32)
            nc.vector.memset(ones128f[:], 1.0)
            one11 = singles.tile([1, 1], bf16)
            nc.vector.memset(one11[:], 1.0)
            ones128b = singles.tile([128, 1], bf16)
            nc.vector.memset(ones128b[:], 1.0)
            warmact = singles.tile([1, 2], f32)
            nc.scalar.activation(
                warmact[0:1, 0:1],
                ones128f[0:1, :],
                mybir.ActivationFunctionType.Tanh,
            )
            nc.scalar.activation(
                warmact[0:1, 1:2],
                ones128f[0:1, :],
                mybir.ActivationFunctionType.Exp,
            )
            zparts = singles.tile([128, GPC], f32)
            pT_sb = singles.tile([128, 4 * GPC], bf16)
            zrecip = singles.tile([GPC, 1], f32)

            from contextlib import ExitStack

            with ExitStack() as stk:
                p_at = stk.enter_context(tc.tile_pool(name="at", bufs=7))
                p_h0 = stk.enter_context(tc.tile_pool(name="h0", bufs=7))
                p_rc = stk.enter_context(tc.tile_pool(name="rc", bufs=7))
                p_hT = stk.enter_context(tc.tile_pool(name="hT", bufs=8))
                p_h = stk.enter_context(tc.tile_pool(name="h", bufs=13))
                p_aggsb = stk.enter_context(tc.tile_pool(name="aggsb", bufs=6))
                p_t = stk.enter_context(tc.tile_pool(name="t", bufs=8))
                p_scr = stk.enter_context(tc.tile_pool(name="scr", bufs=4))
                p_sc = stk.enter_context(tc.tile_pool(name="sc", bufs=14))
                p_pu4 = stk.enter_context(tc.tile_pool(name="pu4", bufs=2))
                p_pqscr = stk.enter_context(tc.tile_pool(name="pqscr", bufs=2))
                ps_a = stk.enter_context(
                    tc.tile_pool(name="ps_a", bufs=3, space="PSUM")
                )
                ps_h = stk.enter_context(
                    tc.tile_pool(name="ps_h", bufs=2, space="PSUM")
                )
                ps_u = stk.enter_context(tc.tile_pool(name="ps_u", bufs=2, space="PSUM"))
                ps_quad = stk.enter_context(
                    tc.tile_pool(name="ps_quad", bufs=1, space="PSUM")
                )
                # hcur[gg] = current h tile [128, 4, F] fp8 (None -> use h0 split)
                hcur = {}
                h0s = {}
                ats = {}
                rcs = {}
                hTs = {}
                cats = {}

                state = {"quad": None, "pcol": None, "pps": [], "qi": 0, "pc": 0}
                pending = []

                def new_quad():
                    return ps_quad.tile([128, D], mybir.dt.float32, name="pquad")

                def pool_begin(ea, eb):
                    (ga, cats_a, attn_a) = ea
                    (gb, cats_b, attn_b) = eb
                    qa, qb = ga % 4, gb % 4
                    if qa == 0:
                        state["quad"] = new_quad()
                        nc.vector.memset(state["quad"][:], 0.0)
                    state["pps"].append((qa, attn_a, cats_a, qb, attn_b, cats_b, state["quad"]))

                def pool_chunk(r):
                    for (qa, attn_a, cats_a, qb, attn_b, cats_b, pooledquad) in state["pps"]:
                        for q, attnCol, cat in (
                            (qa, attn_a, cats_a),
                            (qb, attn_b, cats_b),
                        ):
                            nc.tensor.matmul(
                                pooledquad[32 * q : 32 * q + 1, :],
                                attnCol[:, r : r + 1],
                                cat[:, r, :, :],
                                start=(r == 0),
                                stop=(r == 3),
                                tile_position=(0, 32 * q),
                            )

                def pool_finish():
                    pps = state["pps"]
                    state["pps"] = []
                    for (qa, attn_a, cats_a, qb, attn_b, cats_b, pooledquad) in pps:
                        pool_tail(qb, pooledquad)

                def pool_tail(qb, pooledquad):
                    if qb != 3:
                        return
                    qi = state["qi"]
                    state["qi"] += 1
                    pu4_sb = p_pu4.tile([128, D], bf16, name="pu4_sb")
                    nc.vector.tensor_copy(pu4_sb[:], pooledquad[:])
                    if qi == GPC // 4 - 1:
                        # drain-critical last quad: extract pT columns with
                        # K=1 matmuls on the (idle) PE instead of the serial
                        # XBAR transpose + gather chain
                        pcol_ps = ps_a.tile(
                            [128, D], mybir.dt.float32, tag="a", name="pcol_ps"
                        )
                        for q in range(4):
                            for c in range(4):
                                nc.tensor.matmul(
                                    pcol_ps[:, 4 * c + q : 4 * c + q + 1],
                                    pu4_sb[32 * q : 32 * q + 1, c * 128 : (c + 1) * 128],
                                    ones128b[32 * q : 32 * q + 1, :],
                                    start=(c == 0),
                                    stop=(c == 3),
                                    tile_position=(32 * q, 0),
                                )
                        pcol_sb = p_pqscr.tile(
                            [128, 16], bf16, tag="pcol", name="pcol_sb"
                        )
                        nc.vector.tensor_copy(pcol_sb[:], pcol_ps[:, 0:16])
                        state["pcol"] = pcol_sb
                    else:
                        scr_t = p_pqscr.tile(
                            [128, 4, 128], bf16, tag="pq", name="scr_t"
                        )
                        nc.sync.dma_start_transpose(scr_t[:], pu4_sb[:])
                        for c in range(4):
                            nc.sync.dma_start(
                                pT_sb[:, c * GPC + 4 * qi : c * GPC + 4 * qi + 4],
                                scr_t[:, c, 0:128:32],
                            )

                def emit_pool_pair(ea, eb):
                    pool_begin(ea, eb)
                    for r in range(4):
                        pool_chunk(r)
                    pool_finish()

                def conv_aggT(gg, l):
                    aggT_ps = ps_a.tile([128, D], mybir.dt.float32, tag="a", name="aggT_ps")
                    at_t = ats[gg]
                    if l == 0:
                        # hi/lo split of node features: 4 DoubleRow matmuls
                        h0t = h0s[gg]
                        k = 0
                        for part in range(2):
                            for c in (0, 2):
                                nc.tensor.matmul(
                                    aggT_ps[:],
                                    h0t[:, part, c : c + 2, :],
                                    at_t[:, c : c + 2, :],
                                    start=(k == 0),
                                    stop=(k == 3),
                                    perf_mode=mybir.MatmulPerfMode.DoubleRow,
                                )
                                k += 1
                    else:
                        catg_t, lprev = hcur[gg]
                        for c in (0, 2):
                            nc.tensor.matmul(
                                aggT_ps[:],
                                catg_t[:, c : c + 2, lprev, :],
                                at_t[:, c : c + 2, :],
                                start=(c == 0),
                                stop=(c == 2),
                                perf_mode=mybir.MatmulPerfMode.DoubleRow,
                            )
                    return aggT_ps

                def conv_cast(gg, aggT_ps):
                    aggT_sb = p_aggsb.tile([128, D], bf16, name="aggT_sb")
                    nc.vector.tensor_mul(aggT_sb[:], aggT_ps[:], rcs[gg][:])
                    return aggT_sb

                def conv_linT(gg, l, aggT_sb):
                    linT_ps = ps_a.tile([128, D], mybir.dt.float32, tag="a", name="linT_ps")
                    nc.tensor.matmul(
                        linT_ps[:],
                        convw_sb[:, l, :],
                        aggT_sb[:],
                        start=True,
                        stop=True,
                    )
                    return linT_ps

                def conv_tanh(gg, l, linT_ps):
                    if l == 0:
                        hTs[gg] = p_hT.tile([128, NL, D], mybir.dt.float8e4, tag="hT", name="hTcat")
                    nc.scalar.activation(
                        hTs[gg][:, l, :], linT_ps[:], mybir.ActivationFunctionType.Tanh
                    )

                def conv_transp(gg, l):
                    # fp8 transpose writes PSUM with element step 2
                    h_ps = ps_h.tile([128, 4, 256], mybir.dt.float8e4, name="h_ps")
                    for r in range(4):
                        nc.tensor.transpose(
                            h_ps[:, r, 0:256:2],
                            hTs[gg][:, l, r * 128 : (r + 1) * 128],
                            ident_sb[:],
                        )
                    return h_ps

                def conv_evac(gg, l, h_ps):
                    if l == 0:
                        cats[gg] = p_h.tile(
                            [128, 4, NL, 128], mybir.dt.float8e4, tag="h", name="catg"
                        )
                    nc.vector.tensor_copy(
                        cats[gg][:, :, l, :], h_ps[:, :, 0:256:2]
                    )
                    hcur[gg] = (cats[gg], l)

                def conv_step(gg, l):
                    aggT_ps = conv_aggT(gg, l)
                    aggT_sb = conv_cast(gg, aggT_ps)
                    linT_ps = conv_linT(gg, l, aggT_sb)
                    conv_tanh(gg, l, linT_ps)
                    h_ps = conv_transp(gg, l)
                    conv_evac(gg, l, h_ps)

                def emit_score_stage(prevpair, a):
                    # stage a computes attd-chunk a of uT = attW^T @ cat^T for
                    # both graphs of the previous pair; a==3 finishes scores
                    ppair, tTs = prevpair
                    for gg in ppair:
                        uT_ps = ps_u.tile([128, D], mybir.dt.float32, tag="u", name="uT_ps")
                        for l in (0, 2):
                            nc.tensor.matmul(
                                uT_ps[:],
                                attw_sb[:, l : l + 2, a * 128 : (a + 1) * 128],
                                hTs[gg][:, l : l + 2, :],
                                start=(l == 0),
                                stop=(l == 2),
                                perf_mode=mybir.MatmulPerfMode.DoubleRow,
                            )
                        if a == 0:
                            tTs[gg] = p_t.tile([128, NL, D], mybir.dt.float8e4, tag="t", name="tTcat")
                        nc.scalar.activation(
                            tTs[gg][:, a, :], uT_ps[:], mybir.ActivationFunctionType.Tanh
                        )
                        if a == 3:
                            # s16 = (16 v)^T tT as an fp8 s-row, then transpose
                            # into column layout for exp + pooling
                            s_ps = ps_u.tile([2, D], mybir.dt.float32, tag="u", name="s_ps")
                            for aa in (0, 2):
                                nc.tensor.matmul(
                                    s_ps[:],
                                    vcol_sb[:, aa : aa + 2, 0:2],
                                    tTs[gg][:, aa : aa + 2, :],
                                    start=(aa == 0),
                                    stop=(aa == 2),
                                    perf_mode=mybir.MatmulPerfMode.DoubleRow,
                                )
                            s_sb = p_scr.tile([1, D], bf16, tag="scr", name="s_sb")
                            nc.vector.tensor_copy(s_sb[:], s_ps[0:1, :])
                            sCol_ps = ps_u.tile([128, 8], bf16, tag="u", name="sCol_ps")
                            for r in range(4):
                                nc.tensor.transpose(
                                    sCol_ps[:, 2 * r : 2 * r + 1],
                                    s_sb[0:1, r * 128 : (r + 1) * 128],
                                    one11[:],
                                )
                            attnCol = p_sc.tile([128, 4], mybir.dt.float8e4, tag="ac", name="attnCol")
                            nc.scalar.activation(
                                attnCol[:],
                                sCol_ps[:, 0:8:2],
                                mybir.ActivationFunctionType.Exp,
                                scale=1.0 / 16.0,
                                accum_out=zparts[:, gg : gg + 1],
                            )
                            pending.append((gg, cats[gg], attnCol))

                prev = None
                # first two groups are quads: the fill has no score/pool
                # filler work, so deeper graph-overlap is free there
                groups = [tuple(range(0, 4)), tuple(range(4, 8))] + [
                    tuple(range(g, g + 3)) for g in range(8, GPC, 3)
                ]
                for gi, pair in enumerate(groups):
                    for gg in pair:
                        at_t = p_at.tile([128, NL, D], mybir.dt.float8e4, tag="at")
                        ats[gg] = at_t
                        h0t = p_h0.tile([128, 2, NL, F], mybir.dt.float8e4, tag="h0")
                        h0s[gg] = h0t
                        rc_t = p_rc.tile([128, D], mybir.dt.float32, tag="rc")
                        rcs[gg] = rc_t
                        if gg <= 1:
                            # first pair: split the big at DMA so pieces land
                            # on parallel queues
                            nc.sync.dma_start(at_t[:, 0:2, :], at_d[gg, :, 0:2, :])
                            nc.sync.dma_start(at_t[:, 2:4, :], at_d[gg, :, 2:4, :])
                        else:
                            nc.sync.dma_start(at_t[:], at_d[gg])
                        nc.sync.dma_start(h0t[:], h0_d[gg])
                        nc.sync.dma_start(rc_t[:], recip_d[gg])
                    if gi == 0:
                        nc.sync.dma_start(convw_sb[:], convw_d[:])
                        nc.sync.dma_start(ident_sb[:], ident_d[:])
                        nc.sync.dma_start(attw_sb[:], attw_d[:])
                        nc.sync.dma_start(vcol_sb[:], vcol_d[:])
                        nc.sync.dma_start(outw_sb[:], outw_d[:])

                    # ---------------- conv layers ----------------
                    # (gp==0 runs the same phased loop; prev=None skips the
                    # score stages and pending is empty so no pool work)
                    npool = 1 + (gi % 2)
                    for _ in range(npool):
                        if len(pending) > 3:
                            pool_begin(pending.pop(0), pending.pop(0))
                    for l in range(NL):
                        # phase-interleave the pair so PE never waits on one
                        # graph's cast/tanh chain; score + pool work for older
                        # graphs fills the remaining stalls.  One pool chunk
                        # (2 short matmuls) drips into each layer's evac-wait
                        # gap instead of a single 3us lump on the chain.
                        aps = {gg: conv_aggT(gg, l) for gg in pair}
                        sbs = {gg: conv_cast(gg, aps[gg]) for gg in pair}
                        lps = {gg: conv_linT(gg, l, sbs[gg]) for gg in pair}
                        for gg in pair:
                            conv_tanh(gg, l, lps[gg])
                        if prev is not None:
                            emit_score_stage(prev, l)
                        hps = {gg: conv_transp(gg, l) for gg in pair}
                        for gg in pair:
                            conv_evac(gg, l, hps[gg])
                        if state["pps"]:
                            pool_chunk(l)
                            if l == 3:
                                pool_finish()

                    prev = (pair, {})

                # drain: score the last pair, dripping pool chunks between
                def drain_pool_tick():
                    if not state["pps"] and len(pending) >= 2:
                        pool_begin(pending.pop(0), pending.pop(0))
                        state["pc"] = 0
                    if state["pps"]:
                        pool_chunk(state["pc"])
                        state["pc"] += 1
                        if state["pc"] == 4:
                            pool_finish()

                for r in range(4):
                    emit_score_stage(prev, r)
                    drain_pool_tick()
                    drain_pool_tick()
                while pending or state["pps"]:
                    drain_pool_tick()

                # ---------------- output head ----------------
                pq_fin = new_quad()
                zall_ps = pq_fin[0:GPC, 0:1]
                nc.tensor.matmul(
                    zall_ps, zparts[:], ones128f[:], start=True, stop=True
                )
                nc.vector.reciprocal(zrecip[:], zall_ps)
                # rows 28:31 read the last quad's pcol_sb directly (no pT
                # gather DMAs on the drain path); separate base-0 Z recip
                zallB_ps = pq_fin[0:4, 1:2]
                nc.tensor.matmul(
                    zallB_ps,
                    zparts[:, GPC - 4 : GPC],
                    ones128f[:],
                    start=True,
                    stop=True,
                )
                zrecipB = singles.tile([4, 1], f32)
                nc.vector.reciprocal(zrecipB[:], zallB_ps)
                out_psA = pq_fin[0:28, 2 * OUT : 3 * OUT]
                out_psB = pq_fin[0:4, 3 * OUT : 4 * OUT]
                pcol_fin = state["pcol"]
                for c in range(4):
                    nc.tensor.matmul(
                        out_psA,
                        pT_sb[:, c * GPC : c * GPC + 28],
                        outw_sb[:, c, :],
                        start=(c == 0),
                        stop=(c == 3),
                    )
                for c in range(4):
                    nc.tensor.matmul(
                        out_psB,
                        pcol_fin[:, 4 * c : 4 * c + 4],
                        outw_sb[:, c, :],
                        start=(c == 0),
                        stop=(c == 3),
                    )
                out_finA = singles.tile([GPC, OUT], f32)
                out_finB = singles.tile([4, OUT], f32)
                nc.scalar.activation(
                    out_finA[0:28, :],
                    out_psA,
                    mybir.ActivationFunctionType.Relu,
                    scale=zrecip[0:28, :],
                )
                nc.scalar.activation(
                    out_finB[:],
                    out_psB,
                    mybir.ActivationFunctionType.Relu,
                    scale=zrecipB[:],
                )
                nc.sync.dma_start(out_d[0:28], out_finA[0:28, :])
                nc.sync.dma_start(out_d[28:GPC], out_finB[:])

    nc.compile()
    _NC_CACHE["nc"] = nc
    return nc


def _prep_inputs(node_feat, edge_src, edge_dst, conv_W, att_W, att_v, out_W):
    src = edge_src.astype(np.int64)
    dst = edge_dst.astype(np.int64)
    ls = src - (dst // N) * N  # src local id within dst's graph
    idx = dst * N + ls
    counts = np.bincount(idx, minlength=B * N * N).astype(np.float32)
    A = counts.reshape(B, N, N)
    iN = np.arange(N)
    A[:, iN, iN] += 1.0
    if A.max() > 16:
        raise ValueError("adjacency counts exceed exact fp8 integer range")
    degs = A.sum(axis=2)  # == deg + 1
    At = np.ascontiguousarray(A.transpose(0, 2, 1))  # [g, src, dst]
    at_host = np.ascontiguousarray(
        At.reshape(B, 4, 128, N).transpose(0, 2, 1, 3)
    ).astype(FP8)  # [B, 128, 4, 512] integer counts, exact

    h0 = np.ascontiguousarray(
        node_feat.reshape(B, 4, 128, F).transpose(0, 2, 1, 3)
    )  # [B, 128, 4, 128]
    h0_hi = h0.astype(FP8)
    h0_lo = (h0 - h0_hi.astype(np.float32)).astype(FP8)
    h0_host = np.ascontiguousarray(
        np.stack([h0_hi, h0_lo], axis=2)
    )  # [B, 128, 2, 4, 128]

    recipb = np.ascontiguousarray(
        np.broadcast_to((1.0 / degs)[:, None, :], (B, 128, N))
    ).astype(np.float32)

    convw2 = np.ascontiguousarray(conv_W.transpose(1, 0, 2)).astype(BF16)
    attw2 = (
        np.ascontiguousarray(att_W.reshape(4, 128, D).transpose(1, 0, 2))
    ).astype(FP8)
    vcol = np.zeros((128, NL, 16), dtype=np.float32)
    vcol[:, :, 0] = (att_v * 16.0).reshape(4, 128).T
    vcol = vcol.astype(FP8)
    outw2 = (
        np.ascontiguousarray(out_W.reshape(4, 128, OUT).transpose(1, 0, 2))
    ).astype(BF16)
    ident = np.eye(128, dtype=np.float32).astype(FP8)
    return at_host, h0_host, recipb, convw2, attw2, vcol, outw2, ident


def _host_fallback(node_feat, edge_src, edge_dst, conv_W, conv_b, att_W, att_b,
                   att_v, out_W, out_b):
    # exact numpy mirror of the reference; only used if biases are nonzero
    src = edge_src.astype(np.int64)
    dst = edge_dst.astype(np.int64)
    ls = src - (dst // N) * N
    idx = dst * N + ls
    counts = np.bincount(idx, minlength=B * N * N).astype(np.float32)
    A = counts.reshape(B, N, N)
    iN = np.arange(N)
    A[:, iN, iN] += 1.0
    degs = A.sum(axis=2)[:, :, None]
    h = node_feat.reshape(B, N, F).astype(np.float32)
    cats = []
    for l in range(NL):
        agg = np.matmul(A, h)
        lin = np.matmul(agg, conv_W[l]) + conv_b[l]
        h = np.tanh(lin / degs)
        cats.append(h)
    cat = np.concatenate(cats, axis=2)
    u = np.matmul(cat, att_W) + att_b
    s = np.tanh(u) @ att_v
    s = s - s.max(axis=1, keepdims=True)
    a = np.exp(s)
    a /= a.sum(axis=1, keepdims=True)
    pooled = np.einsum('bn,bnd->bd', a, cat)
    return np.maximum(pooled @ out_W + out_b, 0.0).astype(np.float32)


def kernel(
    node_feat,
    edge_src,
    edge_dst,
    conv_W,
    conv_b,
    att_W,
    att_b,
    att_v,
    out_W,
    out_b,
):
    node_feat = np.asarray(node_feat, dtype=np.float32)
    edge_src = np.asarray(edge_src)
    edge_dst = np.asarray(edge_dst)
    conv_W = np.asarray(conv_W, dtype=np.float32)
    conv_b = np.asarray(conv_b, dtype=np.float32)
    att_W = np.asarray(att_W, dtype=np.float32)
    att_b = np.asarray(att_b, dtype=np.float32)
    att_v = np.asarray(att_v, dtype=np.float32)
    out_W = np.asarray(out_W, dtype=np.float32)
    out_b = np.asarray(out_b, dtype=np.float32)

    if np.any(conv_b) or np.any(att_b) or np.any(out_b):
        return _host_fallback(node_feat, edge_src, edge_dst, conv_W, conv_b,
                              att_W, att_b, att_v, out_W, out_b)

    from concourse.bass_utils import run_bass_kernel_spmd

    at_host, h0_host, recipb, convw2, attw2, vcol, outw2, ident = _prep_inputs(
        node_feat, edge_src, edge_dst, conv_W, att_W, att_v, out_W
    )

    nc = _build_nc()

    in_maps = []
    for c in range(NCORES):
        sl = slice(c * GPC, (c + 1) * GPC)
        in_maps.append({
            "at": at_host[sl],
            "h0": h0_host[sl],
            "recipb": recipb[sl],
            "convw": convw2,
            "attw": attw2,
            "vcol": vcol,
            "outw": outw2,
            "ident": ident,
        })

    res = run_bass_kernel_spmd(nc, in_maps, core_ids=list(range(NCORES)))
    out = np.concatenate([r["out"] for r in res.results], axis=0)
    return np.ascontiguousarray(out.astype(np.float32))
